# revision 7
# baseline (speedup 1.0000x reference)
"""Trainium2 Bass kernel for nn_Decoder (GRU decoder over padded sequences).

Computation (per sample):
  emb = message[:, :T-1] @ W_emb.T + b_emb            (folded into W_c on host)
  xs  = [init_emb, emb]                                (step 0 folded into h1 const)
  GRU over T steps, gather h at lengths-1              (freeze trick: z := 1 past len)
  out = sigmoid(elu(h @ W1.T + b1) @ W2.T + b2)

Sharding: batch data-parallel over 8 cores, host-side length-sort (stratified
round-robin across cores) so each 512-sample chunk only runs to its max length.
"""

import sys

sys.path.insert(0, "/opt/trn_rl_repo")

import numpy as np
import ml_dtypes

import concourse.bacc as bacc
import concourse.mybir as mybir
import concourse.tile as tile
from concourse.bass_utils import run_bass_kernel_spmd

B, T, V, E, H, FC, OUT = 65536, 30, 21, 32, 64, 256, 784
NCORES = 8
BC = B // NCORES          # 8192 samples per core
CH = 512                  # chunk (matmul free dim)
NCH = BC // CH            # 16 chunks
KX = V + 1                # 21 msg rows + 1 freeze-flag row
FREEZE = 40.0             # z-gate preactivation offset for finished samples
OTILES = (OUT + 127) // 128  # 7 output row tiles

USE_BF16 = True
TRACE = False             # set by test harness for profiling
LAST_RESULT = None        # BassKernelResults stash for the harness

_f32 = mybir.dt.float32
_bf16 = mybir.dt.bfloat16


def _sigmoid(x):
    return 1.0 / (1.0 + np.exp(-x))


def _build_nc(step_counts, dt):
    """Build the SPMD kernel. step_counts[k] = number of GRU steps (beyond the
    constant step 0) to run for chunk k — identical on every core."""
    AF = mybir.ActivationFunctionType
    OP = mybir.AluOpType
    nc = bacc.Bacc("TRN2", target_bir_lowering=False, debug=False)

    X = nc.dram_tensor("X", [T - 1, KX, BC], dt, kind="ExternalInput")
    WX = nc.dram_tensor("WX", [KX, 3 * H], dt, kind="ExternalInput")
    WH = nc.dram_tensor("WH", [H, 3 * H], dt, kind="ExternalInput")
    W1T = nc.dram_tensor("W1T", [H, FC], dt, kind="ExternalInput")
    W2T = nc.dram_tensor("W2T", [FC, OUT], dt, kind="ExternalInput")
    BRZ = nc.dram_tensor("BRZ", [2 * H, 1], _f32, kind="ExternalInput")
    BHN = nc.dram_tensor("BHN", [H, 1], _f32, kind="ExternalInput")
    BCN = nc.dram_tensor("BCN", [H, 1], _f32, kind="ExternalInput")
    B1 = nc.dram_tensor("B1", [128, FC // 128], _f32, kind="ExternalInput")
    B2 = nc.dram_tensor("B2", [128, OTILES], _f32, kind="ExternalInput")
    H1 = nc.dram_tensor("H1", [H, 1], _f32, kind="ExternalInput")
    OT = nc.dram_tensor("OT", [OUT, BC], _f32, kind="ExternalOutput")

    with tile.TileContext(nc) as tc:
        with (
            tc.tile_pool(name="weights", bufs=1) as wp,
            tc.tile_pool(name="xin", bufs=2) as xp,
            tc.tile_pool(name="hstate", bufs=2) as hp,
            tc.tile_pool(name="gates", bufs=3) as gp,
            tc.tile_pool(name="head", bufs=3) as fp,
            tc.tile_pool(name="outs", bufs=3) as op_,
            tc.tile_pool(name="psA", bufs=2, space="PSUM") as psA,
            tc.tile_pool(name="psB", bufs=2, space="PSUM") as psB,
            tc.tile_pool(name="psC", bufs=2, space="PSUM") as psC,
        ):
            # --- load weights/biases once ---
            wx = wp.tile([KX, 3 * H], dt)
            nc.sync.dma_start(out=wx[:], in_=WX[:])
            wh = wp.tile([H, 3 * H], dt)
            nc.sync.dma_start(out=wh[:], in_=WH[:])
            w1 = wp.tile([H, FC], dt)
            nc.sync.dma_start(out=w1[:], in_=W1T[:])
            w2a = wp.tile([128, OUT], dt)
            nc.sync.dma_start(out=w2a[:], in_=W2T[0:128, :])
            w2b = wp.tile([128, OUT], dt)
            nc.sync.dma_start(out=w2b[:], in_=W2T[128:256, :])
            brz = wp.tile([2 * H, 1], _f32)
            nc.sync.dma_start(out=brz[:], in_=BRZ[:])
            bhn = wp.tile([H, 1], _f32)
            nc.sync.dma_start(out=bhn[:], in_=BHN[:])
            bcn = wp.tile([H, 1], _f32)
            nc.sync.dma_start(out=bcn[:], in_=BCN[:])
            b1s = wp.tile([128, FC // 128], _f32)
            nc.sync.dma_start(out=b1s[:], in_=B1[:])
            b2s = wp.tile([128, OTILES], _f32)
            nc.sync.dma_start(out=b2s[:], in_=B2[:])
            h1s = wp.tile([H, 1], _f32)
            nc.sync.dma_start(out=h1s[:], in_=H1[:])

            for c in range(NCH):
                nsteps = step_counts[c]
                cs = slice(c * CH, (c + 1) * CH)

                xt = xp.tile([KX, T - 1, CH], dt)
                nc.sync.dma_start(
                    out=xt[:], in_=X[:, :, cs].rearrange("t k b -> k t b")
                )

                # h state ping-pong; init to the constant step-0 output h1
                ha = hp.tile([H, CH], dt, tag="ha")
                hb = hp.tile([H, CH], dt, tag="hb")
                nc.vector.memset(ha[:], 0.0)
                nc.vector.tensor_scalar_add(ha[:], ha[:], h1s[:])

                cur, nxt = ha, hb
                for s in range(1, nsteps + 1):
                    xs_ = xt[:, s - 1, :]
                    # rz preactivation: W_c_rz @ x + W_hh_rz @ h  (PSUM accum)
                    prz = psA.tile([128, CH], _f32, tag="ps")
                    nc.tensor.matmul(prz[:], wx[:, 0:128], xs_, start=True, stop=False)
                    nc.tensor.matmul(prz[:], wh[:, 0:128], cur[:], start=False, stop=True)
                    # n-path: xn and hn separately. The [64:128] halves of these
                    # banks are scratch for u/v (PSUM operands are exempt from
                    # the SBUF same-start-partition rule).
                    pxnF = psB.tile([128, CH], _f32, tag="pxn")
                    pxn = pxnF[0:H, :]
                    nc.tensor.matmul(pxn, wx[:, 128:192], xs_, start=True, stop=True)
                    phnF = psC.tile([128, CH], _f32, tag="phn")
                    phn = phnF[0:H, :]
                    nc.tensor.matmul(phn, wh[:, 128:192], cur[:], start=True, stop=True)

                    rz = gp.tile([128, CH], dt, tag="rz")
                    nc.scalar.activation(rz[:], prz[:], AF.Sigmoid, bias=brz[:])
                    t1 = gp.tile([H, CH], dt, tag="t1")
                    # (hn + b_hn) * r
                    nc.vector.scalar_tensor_tensor(
                        t1[:], phn, bhn[:], rz[0:H, :], op0=OP.add, op1=OP.mult
                    )
                    t2 = gp.tile([H, CH], dt, tag="t2")
                    # (xn + b_cn) + t1
                    nc.vector.scalar_tensor_tensor(
                        t2[:], pxn, bcn[:], t1[:], op0=OP.add, op1=OP.add
                    )
                    nt = gp.tile([H, CH], dt, tag="nt")
                    nc.scalar.activation(nt[:], t2[:], AF.Tanh)
                    # h' = n + z * (h - n);  u, v staged in PSUM half-banks so
                    # every op's SBUF operands share a start partition
                    u = pxnF[H:128, :]
                    nc.vector.tensor_sub(u, cur[:], nt[:])
                    v = phnF[H:128, :]
                    nc.vector.tensor_mul(v, rz[H : 2 * H, :], u)
                    nc.vector.tensor_add(nxt[:], nt[:], v)
                    cur, nxt = nxt, cur

                # --- MLP head on final h ---
                f2rhs = []
                for j in range(FC // 128):
                    pf = psA.tile([128, CH], _f32, tag="ps")
                    nc.tensor.matmul(
                        pf[:], w1[:, j * 128 : (j + 1) * 128], cur[:], start=True, stop=True
                    )
                    # elu(x+b1) + 1 == (x+b1 - m) + exp(m),  m = min(x+b1, 0)
                    m = fp.tile([128, CH], dt, tag="m")
                    nc.vector.tensor_scalar(
                        m[:], pf[:], b1s[:, j : j + 1], 0.0, op0=OP.add, op1=OP.min
                    )
                    e = fp.tile([128, CH], dt, tag="e")
                    nc.scalar.activation(e[:], m[:], AF.Exp)
                    p = fp.tile([128, CH], dt, tag="p")
                    nc.vector.scalar_tensor_tensor(
                        p[:], pf[:], b1s[:, j : j + 1], m[:], op0=OP.add, op1=OP.subtract
                    )
                    fr = fp.tile([128, CH], dt, tag=f"fr{j}")
                    nc.vector.tensor_add(fr[:], p[:], e[:])
                    f2rhs.append(fr)

                for mt in range(OTILES):
                    mw = min(128, OUT - mt * 128)
                    po_full = psA.tile([128, CH], _f32, tag="ps")
                    po = po_full[0:mw, :]
                    nc.tensor.matmul(
                        po[:],
                        w2a[:, mt * 128 : mt * 128 + mw],
                        f2rhs[0][:],
                        start=True,
                        stop=False,
                    )
                    nc.tensor.matmul(
                        po[:],
                        w2b[:, mt * 128 : mt * 128 + mw],
                        f2rhs[1][:],
                        start=False,
                        stop=True,
                    )
                    ot = op_.tile([mw, CH], _f32, tag="ot")
                    nc.scalar.activation(
                        ot[:], po[:], AF.Sigmoid, bias=b2s[0:mw, mt : mt + 1]
                    )
                    nc.sync.dma_start(
                        out=OT[mt * 128 : mt * 128 + mw, cs], in_=ot[:]
                    )

    nc.compile()
    return nc


def kernel(message, lengths, init_emb, W_emb, b_emb, W_ih, W_hh, b_ih, b_hh,
           W1, b1, W2, b2):
    global LAST_RESULT
    message = np.asarray(message, dtype=np.float32)
    lengths = np.asarray(lengths).astype(np.int64)
    f8 = np.float64
    np_dt = ml_dtypes.bfloat16 if USE_BF16 else np.float32
    dt = _bf16 if USE_BF16 else _f32

    # --- fold embedding into input weights;  step 0 is a constant ---
    W_c = W_ih.astype(f8) @ W_emb.astype(f8)                # [3H, V]
    b_c = W_ih.astype(f8) @ b_emb.astype(f8) + b_ih         # [3H]
    gx0 = W_ih.astype(f8) @ init_emb.astype(f8) + b_ih
    gh0 = b_hh.astype(f8)
    r0 = _sigmoid(gx0[:H] + gh0[:H])
    z0 = _sigmoid(gx0[H : 2 * H] + gh0[H : 2 * H])
    n0 = np.tanh(gx0[2 * H :] + r0 * gh0[2 * H :])
    h1 = (1.0 - z0) * n0                                    # h after step 0

    # --- length-sort, stratify across cores ---
    perm = np.argsort(lengths, kind="stable")
    lsort = lengths[perm]
    # chunk k (on every core) covers global sorted ranks [k*4096, (k+1)*4096)
    step_counts = [int(lsort[min((k + 1) * CH * NCORES, B) - 1]) - 1 for k in range(NCH)]

    # --- shared weight tensors ---
    WXh = np.zeros((KX, 3 * H), f8)
    WXh[:V] = W_c.T
    WXh[V, H : 2 * H] = FREEZE
    wxd = WXh.astype(np_dt)
    whd = np.ascontiguousarray(W_hh.T).astype(np_dt)
    w1d = np.ascontiguousarray(W1.T).astype(np_dt)
    w2d = np.ascontiguousarray(W2.T).astype(np_dt)
    brzd = np.ascontiguousarray((b_c[: 2 * H] + b_hh[: 2 * H]).astype(np.float32).reshape(2 * H, 1))
    bhnd = np.ascontiguousarray(b_hh[2 * H :].astype(np.float32).reshape(H, 1))
    bcnd = np.ascontiguousarray(b_c[2 * H :].astype(np.float32).reshape(H, 1))
    b1d = np.ascontiguousarray(np.asarray(b1, np.float32).reshape(FC // 128, 128).T)
    b2f = (np.asarray(b2, f8) - W2.astype(f8).sum(axis=1)).astype(np.float32)
    b2p = np.zeros(OTILES * 128, np.float32)
    b2p[:OUT] = b2f
    b2d = np.ascontiguousarray(b2p.reshape(OTILES, 128).T)
    h1d = np.ascontiguousarray(h1.astype(np.float32).reshape(H, 1))

    # --- per-core inputs ---
    trange = np.arange(T - 1)
    in_maps = []
    core_idx = []
    for c in range(NCORES):
        ic = perm[c::NCORES]
        core_idx.append(ic)
        mc = message[ic][:, : T - 1, :]                     # [BC, 29, 21]
        Xc = np.empty((T - 1, KX, BC), dtype=np_dt)
        Xc[:, :V, :] = mc.transpose(1, 2, 0).astype(np_dt)
        Xc[:, V, :] = (lengths[ic][None, :] <= trange[:, None] + 1).astype(np_dt)
        in_maps.append({
            "X": Xc, "WX": wxd, "WH": whd, "W1T": w1d, "W2T": w2d,
            "BRZ": brzd, "BHN": bhnd, "BCN": bcnd, "B1": b1d, "B2": b2d,
            "H1": h1d,
        })

    nc = _build_nc(step_counts, dt)
    res = run_bass_kernel_spmd(nc, in_maps, core_ids=list(range(NCORES)), trace=TRACE)
    LAST_RESULT = res

    out = np.empty((B, OUT), np.float32)
    for c in range(NCORES):
        out[core_idx[c]] = res.results[c]["OT"].T
    return out


# revision 12
# speedup vs baseline: 1.2430x; 1.2430x over previous
"""Trainium2 Bass kernel for nn_Decoder (GRU decoder over padded sequences).

Computation (per sample):
  emb = message[:, :T-1] @ W_emb.T + b_emb            (folded into W_c on host)
  xs  = [init_emb, emb]                                (step 0 folded into h1 const)
  GRU over T steps, gather h at lengths-1              (freeze trick: z := 1 past len)
  out = sigmoid(elu(h @ W1.T + b1) @ W2.T + b2)

Sharding: batch data-parallel over 8 cores, host-side length-sort (stratified
round-robin across cores) so each chunk only runs to its max length.

Device layout: features on partitions, batch on the free dim. The GRU state h
(and the weights it multiplies) live at partition base 64 so that the z-gate
slice of the sigmoid output tile (partitions 64..127) is base-aligned with the
h-update operands — every elementwise op's SBUF operands share a start
partition, which the ISA requires. Chunks are processed in pairs (1024
samples) with the h-update ops fused at FD=1024, and pairs are interleaved
PIPE at a time so the serial per-step dependency chain overlaps.
"""

import sys

sys.path.insert(0, "/opt/trn_rl_repo")

import numpy as np
import ml_dtypes

import concourse.bacc as bacc
import concourse.mybir as mybir
import concourse.tile as tile
from concourse.bass_utils import run_bass_kernel_spmd

B, T, V, E, H, FC, OUT = 65536, 30, 21, 32, 64, 256, 784
NCORES = 8
BC = B // NCORES          # 8192 samples per core
CH = 512                  # matmul free dim (one PSUM bank)
CH2 = 2 * CH              # pair width
NP = BC // CH2            # 8 pairs per core
FREEZE = 40.0             # z-gate preactivation offset for finished samples
KX = V + 1                # 21 msg rows + 1 freeze-flag row
OTILES = (OUT + 127) // 128  # 7 output row tiles
PIPE = 2                  # pairs in flight
XB = 8                    # GRU steps per streamed X block

USE_BF16 = True
GP_OPS = True             # h-update elementwise on GpSimd (else VectorE)
TRACE = False             # set by test harness for profiling
LAST_RESULT = None        # BassKernelResults stash for the harness

_f32 = mybir.dt.float32
_bf16 = mybir.dt.bfloat16


def _sigmoid(x):
    return 1.0 / (1.0 + np.exp(-x))


def _build_nc(pair_steps, dt):
    """Build the SPMD kernel. pair_steps[p] = GRU steps (beyond the constant
    step 0) for pair p — identical on every core."""
    AF = mybir.ActivationFunctionType
    OP = mybir.AluOpType
    nc = bacc.Bacc("TRN2", target_bir_lowering=False, debug=False)

    X = nc.dram_tensor("X", [T - 1, KX, BC], dt, kind="ExternalInput")
    WX = nc.dram_tensor("WX", [KX, 3 * H], dt, kind="ExternalInput")
    WH = nc.dram_tensor("WH", [H, 3 * H], dt, kind="ExternalInput")
    W1T = nc.dram_tensor("W1T", [H, FC], dt, kind="ExternalInput")
    W2T = nc.dram_tensor("W2T", [FC, OUT], dt, kind="ExternalInput")
    BRZ = nc.dram_tensor("BRZ", [2 * H, 1], _f32, kind="ExternalInput")
    BHN = nc.dram_tensor("BHN", [H, 1], _f32, kind="ExternalInput")
    BCN = nc.dram_tensor("BCN", [H, 1], _f32, kind="ExternalInput")
    B1 = nc.dram_tensor("B1", [128, FC // 128], _f32, kind="ExternalInput")
    B2 = nc.dram_tensor("B2", [128, OTILES], _f32, kind="ExternalInput")
    H1 = nc.dram_tensor("H1", [H, 1], _f32, kind="ExternalInput")
    OT = nc.dram_tensor("OT", [OUT, BC], _f32, kind="ExternalOutput")

    ew = nc.gpsimd if GP_OPS else nc.vector

    with tile.TileContext(nc) as tc:
        with (
            tc.tile_pool(name="weights", bufs=1) as wp,
            tc.tile_pool(name="xin", bufs=2 * PIPE) as xp,
            tc.tile_pool(name="hstate", bufs=2 * PIPE) as hp,
            tc.tile_pool(name="hfinal", bufs=1) as hf,
            tc.tile_pool(name="gates", bufs=2 * PIPE) as gp,
            tc.tile_pool(name="head", bufs=3) as fp,
            tc.tile_pool(name="outs", bufs=3) as op_,
            tc.tile_pool(name="psA", bufs=3, space="PSUM") as psA,
            tc.tile_pool(name="psB", bufs=3, space="PSUM") as psB,
            tc.tile_pool(name="psC", bufs=2, space="PSUM") as psC,
        ):
            # --- load weights/biases once; h-side weights live at base 64 ---
            wx = wp.tile([KX, 3 * H], dt)
            nc.sync.dma_start(out=wx[:], in_=WX[:])
            whF = wp.tile([128, 3 * H], dt)
            nc.sync.dma_start(out=whF[H:128, :], in_=WH[:])
            w1F = wp.tile([128, FC], dt)
            nc.sync.dma_start(out=w1F[H:128, :], in_=W1T[:])
            w2a = wp.tile([128, OUT], dt)
            nc.sync.dma_start(out=w2a[:], in_=W2T[0:128, :])
            w2b = wp.tile([128, OUT], dt)
            nc.sync.dma_start(out=w2b[:], in_=W2T[128:256, :])
            brz = wp.tile([2 * H, 1], _f32)
            nc.sync.dma_start(out=brz[:], in_=BRZ[:])
            bhn = wp.tile([H, 1], _f32)
            nc.sync.dma_start(out=bhn[:], in_=BHN[:])
            bcn = wp.tile([H, 1], _f32)
            nc.sync.dma_start(out=bcn[:], in_=BCN[:])
            b1s = wp.tile([128, FC // 128], _f32)
            nc.sync.dma_start(out=b1s[:], in_=B1[:])
            b2s = wp.tile([128, OTILES], _f32)
            nc.sync.dma_start(out=b2s[:], in_=B2[:])
            h1F = wp.tile([128, 1], _f32)
            nc.sync.dma_start(out=h1F[H:128, :], in_=H1[:])

            wh = whF[H:128, :]
            hfin = []

            def start_pair(p):
                """Allocate tiles + init h for pair p; returns state dict."""
                ha = hp.tile([128, CH2], dt, tag="hpi")
                hb = hp.tile([128, CH2], dt, tag="hpo")
                hfp = hf.tile([128, CH2], dt, tag=f"hf{p}")
                hfin.append(hfp)
                nc.vector.memset(ha[H:128, :], 0.0)
                nc.vector.tensor_scalar_add(ha[H:128, :], ha[H:128, :], h1F[H:128, :])
                if pair_steps[p] == 0:
                    nc.vector.memset(hfp[H:128, :], 0.0)
                    nc.vector.tensor_scalar_add(
                        hfp[H:128, :], hfp[H:128, :], h1F[H:128, :]
                    )
                return {"xt": None, "cur": ha, "nxt": hb, "hf": hfp, "p": p}

            def emit_step(st, s):
                """One GRU step for both chunks of a pair."""
                cur = st["cur"]
                nsteps = pair_steps[st["p"]]
                if (s - 1) % XB == 0:
                    nb = min(XB, nsteps - (s - 1))
                    xtn = xp.tile([KX, XB, CH2], dt, tag="xt")
                    st["xt"] = xtn
                    ps = slice(st["p"] * CH2, (st["p"] + 1) * CH2)
                    nc.sync.dma_start(
                        out=st["xt"][:, 0:nb, :],
                        in_=X[s - 1 : s - 1 + nb, :, ps].rearrange("t k b -> k t b"),
                    )
                xt = st["xt"]
                rzF = gp.tile([128, CH2], dt, tag="rz")
                t2F = gp.tile([H, CH2], dt, tag="t2")
                for g in (0, 1):
                    gs = slice(g * CH, (g + 1) * CH)
                    xs_ = xt[:, (s - 1) % XB, gs]
                    cur_g = cur[H:128, gs]
                    prz = psA.tile([128, CH], _f32, tag="ps")
                    nc.tensor.matmul(prz[:], wx[:, 0:128], xs_, start=True, stop=False)
                    nc.tensor.matmul(prz[:], wh[:, 0:128], cur_g, start=False, stop=True)
                    pxn = psB.tile([H, CH], _f32, tag="pxn")
                    nc.tensor.matmul(pxn[:], wx[:, 128:192], xs_, start=True, stop=True)
                    phn = psC.tile([H, CH], _f32, tag="phn")
                    nc.tensor.matmul(phn[:], wh[:, 128:192], cur_g, start=True, stop=True)
                    nc.scalar.activation(rzF[:, gs], prz[:], AF.Sigmoid, bias=brz[:])
                    t1 = gp.tile([H, CH], dt, tag="t1")
                    # (hn + b_hn) * r
                    nc.vector.scalar_tensor_tensor(
                        t1[:], phn[:], bhn[:], rzF[0:H, gs], op0=OP.add, op1=OP.mult
                    )
                    # (xn + b_cn) + t1
                    nc.vector.scalar_tensor_tensor(
                        t2F[:, gs], pxn[:], bcn[:], t1[:], op0=OP.add, op1=OP.add
                    )
                # joint FD=1024 tail:  n = tanh(t2);  h' = n + z*(h - n)
                ntF = gp.tile([128, CH2], dt, tag="nt")
                nc.scalar.activation(ntF[H:128, :], t2F[:], AF.Tanh)
                uF = gp.tile([128, CH2], dt, tag="u")
                ew.tensor_sub(uF[H:128, :], cur[H:128, :], ntF[H:128, :])
                vF = gp.tile([128, CH2], dt, tag="v")
                ew.tensor_mul(vF[H:128, :], rzF[H:128, :], uF[H:128, :])
                dst = st["hf"] if s == nsteps else st["nxt"]
                ew.tensor_add(dst[H:128, :], ntF[H:128, :], vF[H:128, :])
                st["cur"], st["nxt"] = dst, st["cur"]

            # --- GRU loops: PIPE pairs interleaved ---
            for base in range(0, NP, PIPE):
                grp = [start_pair(p) for p in range(base, min(base + PIPE, NP))]
                maxs = max(pair_steps[st["p"]] for st in grp)
                for s in range(1, maxs + 1):
                    for st in grp:
                        if s <= pair_steps[st["p"]]:
                            emit_step(st, s)

            # --- MLP heads (deferred: avoids ACT table swaps mid-loop) ---
            for p in range(NP):
                for g in (0, 1):
                    gs = slice(g * CH, (g + 1) * CH)
                    cur_g = hfin[p][H:128, gs]
                    f2rhs = []
                    for j in range(FC // 128):
                        pf = psA.tile([128, CH], _f32, tag="ps")
                        nc.tensor.matmul(
                            pf[:], w1F[H:128, j * 128 : (j + 1) * 128], cur_g,
                            start=True, stop=True,
                        )
                        # elu(x+b1) + 1 == (x+b1 - m) + exp(m),  m = min(x+b1, 0)
                        m = fp.tile([128, CH], dt, tag="m")
                        nc.vector.tensor_scalar(
                            m[:], pf[:], b1s[:, j : j + 1], 0.0, op0=OP.add, op1=OP.min
                        )
                        e = fp.tile([128, CH], dt, tag="e")
                        nc.scalar.activation(e[:], m[:], AF.Exp)
                        pp = fp.tile([128, CH], dt, tag="pp")
                        nc.vector.scalar_tensor_tensor(
                            pp[:], pf[:], b1s[:, j : j + 1], m[:],
                            op0=OP.add, op1=OP.subtract,
                        )
                        fr = fp.tile([128, CH], dt, tag=f"fr{j}")
                        nc.vector.tensor_add(fr[:], pp[:], e[:])
                        f2rhs.append(fr)

                    for mt in range(OTILES):
                        mw = min(128, OUT - mt * 128)
                        poF = psC.tile([128, CH], _f32, tag="phn")
                        po = poF[0:mw, :]
                        nc.tensor.matmul(
                            po, w2a[:, mt * 128 : mt * 128 + mw], f2rhs[0][:],
                            start=True, stop=False,
                        )
                        nc.tensor.matmul(
                            po, w2b[:, mt * 128 : mt * 128 + mw], f2rhs[1][:],
                            start=False, stop=True,
                        )
                        ot = op_.tile([mw, CH], _f32, tag="ot")
                        nc.scalar.activation(
                            ot[:], po, AF.Sigmoid, bias=b2s[0:mw, mt : mt + 1]
                        )
                        nc.sync.dma_start(
                            out=OT[mt * 128 : mt * 128 + mw, p * CH2 + g * CH :
                                   p * CH2 + (g + 1) * CH],
                            in_=ot[:],
                        )

    nc.compile()
    return nc


def kernel(message, lengths, init_emb, W_emb, b_emb, W_ih, W_hh, b_ih, b_hh,
           W1, b1, W2, b2):
    global LAST_RESULT
    message = np.asarray(message, dtype=np.float32)
    lengths = np.asarray(lengths).astype(np.int64)
    f8 = np.float64
    np_dt = ml_dtypes.bfloat16 if USE_BF16 else np.float32
    dt = _bf16 if USE_BF16 else _f32

    # --- fold embedding into input weights;  step 0 is a constant ---
    W_c = W_ih.astype(f8) @ W_emb.astype(f8)                # [3H, V]
    b_c = W_ih.astype(f8) @ b_emb.astype(f8) + b_ih         # [3H]
    gx0 = W_ih.astype(f8) @ init_emb.astype(f8) + b_ih
    gh0 = b_hh.astype(f8)
    r0 = _sigmoid(gx0[:H] + gh0[:H])
    z0 = _sigmoid(gx0[H : 2 * H] + gh0[H : 2 * H])
    n0 = np.tanh(gx0[2 * H :] + r0 * gh0[2 * H :])
    h1 = (1.0 - z0) * n0                                    # h after step 0

    # --- length-sort, stratify across cores ---
    perm = np.argsort(lengths, kind="stable")
    lsort = lengths[perm]
    # pair p (on every core) covers global sorted ranks [p*8192, (p+1)*8192)
    pair_steps = [int(lsort[min((p + 1) * CH2 * NCORES, B) - 1]) - 1
                  for p in range(NP)]

    # --- shared weight tensors ---
    WXh = np.zeros((KX, 3 * H), f8)
    WXh[:V] = W_c.T
    WXh[V, H : 2 * H] = FREEZE
    wxd = WXh.astype(np_dt)
    whd = np.ascontiguousarray(W_hh.T).astype(np_dt)
    w1d = np.ascontiguousarray(W1.T).astype(np_dt)
    w2d = np.ascontiguousarray(W2.T).astype(np_dt)
    brzd = np.ascontiguousarray((b_c[: 2 * H] + b_hh[: 2 * H]).astype(np.float32).reshape(2 * H, 1))
    bhnd = np.ascontiguousarray(b_hh[2 * H :].astype(np.float32).reshape(H, 1))
    bcnd = np.ascontiguousarray(b_c[2 * H :].astype(np.float32).reshape(H, 1))
    b1d = np.ascontiguousarray(np.asarray(b1, np.float32).reshape(FC // 128, 128).T)
    b2f = (np.asarray(b2, f8) - W2.astype(f8).sum(axis=1)).astype(np.float32)
    b2p = np.zeros(OTILES * 128, np.float32)
    b2p[:OUT] = b2f
    b2d = np.ascontiguousarray(b2p.reshape(OTILES, 128).T)
    h1d = np.ascontiguousarray(h1.astype(np.float32).reshape(H, 1))

    # --- per-core inputs ---
    trange = np.arange(T - 1)
    in_maps = []
    core_idx = []
    for c in range(NCORES):
        ic = perm[c::NCORES]
        core_idx.append(ic)
        mc = message[ic][:, : T - 1, :]                     # [BC, 29, 21]
        Xc = np.empty((T - 1, KX, BC), dtype=np_dt)
        Xc[:, :V, :] = mc.transpose(1, 2, 0).astype(np_dt)
        Xc[:, V, :] = (lengths[ic][None, :] <= trange[:, None] + 1).astype(np_dt)
        in_maps.append({
            "X": Xc, "WX": wxd, "WH": whd, "W1T": w1d, "W2T": w2d,
            "BRZ": brzd, "BHN": bhnd, "BCN": bcnd, "B1": b1d, "B2": b2d,
            "H1": h1d,
        })

    nc = _build_nc(pair_steps, dt)
    res = run_bass_kernel_spmd(nc, in_maps, core_ids=list(range(NCORES)), trace=TRACE)
    LAST_RESULT = res

    out = np.empty((B, OUT), np.float32)
    for c in range(NCORES):
        out[core_idx[c]] = res.results[c]["OT"].T
    return out


# revision 16
# speedup vs baseline: 1.3068x; 1.0514x over previous
"""Trainium2 Bass kernel for nn_Decoder (GRU decoder over padded sequences).

Computation (per sample):
  emb = message[:, :T-1] @ W_emb.T + b_emb            (folded into W_c on host)
  xs  = [init_emb, emb]                                (step 0 folded into h1 const)
  GRU over T steps, gather h at lengths-1              (freeze trick: z := 1 past len)
  out = sigmoid(elu(h @ W1.T + b1) @ W2.T + b2)

Sharding: batch data-parallel over 8 cores, host-side length-sort (stratified
round-robin across cores) so each chunk only runs to its max length.

Device layout: features on partitions, batch on the free dim. The GRU state h
(and the weights it multiplies) live at partition base 64 so that the z-gate
slice of the sigmoid output tile (partitions 64..127) is base-aligned with the
h-update operands — every elementwise op's SBUF operands share a start
partition, which the ISA requires. Chunks are processed in pairs (1024
samples) with the h-update ops fused at FD=1024, and pairs are interleaved
PIPE at a time so the serial per-step dependency chain overlaps.
"""

import sys

sys.path.insert(0, "/opt/trn_rl_repo")

import numpy as np
import ml_dtypes

import concourse.bacc as bacc
import concourse.mybir as mybir
import concourse.tile as tile
from concourse.bass_utils import run_bass_kernel_spmd

B, T, V, E, H, FC, OUT = 65536, 30, 21, 32, 64, 256, 784
NCORES = 8
BC = B // NCORES          # 8192 samples per core
CH = 512                  # matmul free dim (one PSUM bank)
CH2 = 2 * CH              # pair width
NP = BC // CH2            # 8 pairs per core
FREEZE = 40.0             # z-gate preactivation offset for finished samples
KX = V + 1                # 21 msg rows + 1 freeze-flag row
OTILES = (OUT + 127) // 128  # 7 output row tiles
PIPE = 2                  # pairs in flight
XB = 8                    # GRU steps per streamed X block

USE_BF16 = True
GP_HP = True              # final h-update add on GpSimd (else VectorE)
HN_COPY = True            # stage hn PSUM->SBUF via ScalarE so t1 runs at 2x
TRACE = False             # set by test harness for profiling
LAST_RESULT = None        # BassKernelResults stash for the harness

_f32 = mybir.dt.float32
_bf16 = mybir.dt.bfloat16


def _sigmoid(x):
    return 1.0 / (1.0 + np.exp(-x))


def _build_nc(pair_steps, dt):
    """Build the SPMD kernel. pair_steps[p] = GRU steps (beyond the constant
    step 0) for pair p — identical on every core."""
    AF = mybir.ActivationFunctionType
    OP = mybir.AluOpType
    nc = bacc.Bacc("TRN2", target_bir_lowering=False, debug=False)

    X = nc.dram_tensor("X", [T - 1, KX, BC], dt, kind="ExternalInput")
    WX = nc.dram_tensor("WX", [KX, 3 * H], dt, kind="ExternalInput")
    WH = nc.dram_tensor("WH", [H, 3 * H], dt, kind="ExternalInput")
    W1T = nc.dram_tensor("W1T", [H, FC], dt, kind="ExternalInput")
    W2T = nc.dram_tensor("W2T", [FC, OUT], dt, kind="ExternalInput")
    BRZ = nc.dram_tensor("BRZ", [2 * H, 1], _f32, kind="ExternalInput")
    BHN = nc.dram_tensor("BHN", [H, 1], _f32, kind="ExternalInput")
    BCN = nc.dram_tensor("BCN", [H, 1], _f32, kind="ExternalInput")
    B1 = nc.dram_tensor("B1", [128, FC // 128], _f32, kind="ExternalInput")
    B2 = nc.dram_tensor("B2", [128, OTILES], _f32, kind="ExternalInput")
    H1 = nc.dram_tensor("H1", [H, 1], _f32, kind="ExternalInput")
    OT = nc.dram_tensor("OT", [OUT, BC], _f32, kind="ExternalOutput")

    ew = nc.gpsimd if GP_HP else nc.vector

    with tile.TileContext(nc) as tc:
        with (
            tc.tile_pool(name="weights", bufs=1) as wp,
            tc.tile_pool(name="xin", bufs=2 * PIPE) as xp,
            tc.tile_pool(name="hstate", bufs=2 * PIPE) as hp,
            tc.tile_pool(name="hfinal", bufs=1) as hf,
            tc.tile_pool(name="gates", bufs=3) as gp,
            tc.tile_pool(name="head", bufs=3) as fp,
            tc.tile_pool(name="frhs", bufs=1) as frp,
            tc.tile_pool(name="outs", bufs=3) as op_,
            tc.tile_pool(name="psA", bufs=2, space="PSUM") as psA,
            tc.tile_pool(name="psB", bufs=2, space="PSUM") as psB,
        ):
            # --- load weights/biases once; h-side weights live at base 64 ---
            wx = wp.tile([KX, 3 * H], dt)
            nc.sync.dma_start(out=wx[:], in_=WX[:])
            whF = wp.tile([128, 3 * H], dt)
            nc.sync.dma_start(out=whF[H:128, :], in_=WH[:])
            w1F = wp.tile([128, FC], dt)
            nc.sync.dma_start(out=w1F[H:128, :], in_=W1T[:])
            w2a = wp.tile([128, OUT], dt)
            nc.sync.dma_start(out=w2a[:], in_=W2T[0:128, :])
            w2b = wp.tile([128, OUT], dt)
            nc.sync.dma_start(out=w2b[:], in_=W2T[128:256, :])
            brz = wp.tile([2 * H, 1], _f32)
            nc.sync.dma_start(out=brz[:], in_=BRZ[:])
            bhn = wp.tile([H, 1], _f32)
            nc.sync.dma_start(out=bhn[:], in_=BHN[:])
            bcn = wp.tile([H, 1], _f32)
            nc.sync.dma_start(out=bcn[:], in_=BCN[:])
            b1s = wp.tile([128, FC // 128], _f32)
            nc.sync.dma_start(out=b1s[:], in_=B1[:])
            b2s = wp.tile([128, OTILES], _f32)
            nc.sync.dma_start(out=b2s[:], in_=B2[:])
            h1F = wp.tile([128, 1], _f32)
            nc.sync.dma_start(out=h1F[H:128, :], in_=H1[:])

            wh = whF[H:128, :]
            hfin = []

            def start_pair(p):
                """Allocate tiles + init h for pair p; returns state dict."""
                ha = hp.tile([128, CH2], dt, tag="hpi")
                hb = hp.tile([128, CH2], dt, tag="hpo")
                hfp = hf.tile([128, CH2], dt, tag=f"hf{p}")
                hfin.append(hfp)
                nc.vector.memset(ha[H:128, :], 0.0)
                nc.vector.tensor_scalar_add(ha[H:128, :], ha[H:128, :], h1F[H:128, :])
                if pair_steps[p] == 0:
                    nc.vector.memset(hfp[H:128, :], 0.0)
                    nc.vector.tensor_scalar_add(
                        hfp[H:128, :], hfp[H:128, :], h1F[H:128, :]
                    )
                return {"xt": None, "cur": ha, "nxt": hb, "hf": hfp, "p": p}

            def emit_step(st, s):
                """One GRU step for both chunks of a pair."""
                cur = st["cur"]
                nsteps = pair_steps[st["p"]]
                if (s - 1) % XB == 0:
                    nb = min(XB, nsteps - (s - 1))
                    xtn = xp.tile([KX, XB, CH2], dt, tag="xt")
                    st["xt"] = xtn
                    ps = slice(st["p"] * CH2, (st["p"] + 1) * CH2)
                    nc.sync.dma_start(
                        out=st["xt"][:, 0:nb, :],
                        in_=X[s - 1 : s - 1 + nb, :, ps].rearrange("t k b -> k t b"),
                    )
                xt = st["xt"]
                # 2-bank PSUM tiles: both chunks side by side -> FD=1024 ops.
                # pn layout: hn on partitions 0..63, xn on partitions 64..127.
                prz2 = psA.tile([128, CH2], _f32, tag="ps")
                pn2 = psB.tile([128, CH2], _f32, tag="pn")
                for g in (0, 1):
                    gs = slice(g * CH, (g + 1) * CH)
                    xs_ = xt[:, (s - 1) % XB, gs]
                    cur_g = cur[H:128, gs]
                    nc.tensor.matmul(prz2[:, gs], wx[:, 0:128], xs_, start=True, stop=False)
                    nc.tensor.matmul(prz2[:, gs], wh[:, 0:128], cur_g, start=False, stop=True)
                    nc.tensor.matmul(pn2[0:H, gs], wh[:, 128:192], cur_g, start=True, stop=True)
                    nc.tensor.matmul(pn2[H:128, gs], wx[:, 128:192], xs_, start=True, stop=True)
                rzF = gp.tile([128, CH2], dt, tag="rz")
                nc.scalar.activation(rzF[:], prz2[:], AF.Sigmoid, bias=brz[:])
                t1 = gp.tile([H, CH2], dt, tag="t1")
                if HN_COPY:
                    hns = gp.tile([H, CH2], dt, tag="hns")
                    nc.scalar.copy(hns[:], pn2[0:H, :])
                    # (hn + b_hn) * r   (all-SBUF bf16 -> 2x mode)
                    nc.vector.scalar_tensor_tensor(
                        t1[:], hns[:], bhn[:], rzF[0:H, :], op0=OP.add, op1=OP.mult
                    )
                else:
                    nc.vector.scalar_tensor_tensor(
                        t1[:], pn2[0:H, :], bhn[:], rzF[0:H, :], op0=OP.add, op1=OP.mult
                    )
                # (xn + b_cn) + t1   (xn read from PSUM at base 64: exempt)
                t2F = gp.tile([H, CH2], dt, tag="t2")
                nc.vector.scalar_tensor_tensor(
                    t2F[:], pn2[H:128, :], bcn[:], t1[:], op0=OP.add, op1=OP.add
                )
                # n = tanh(t2);  h' = n + z*(h - n)
                ntF = gp.tile([128, CH2], dt, tag="nt")
                nc.scalar.activation(ntF[H:128, :], t2F[:], AF.Tanh)
                uF = gp.tile([128, CH2], dt, tag="u")
                nc.vector.tensor_sub(uF[H:128, :], cur[H:128, :], ntF[H:128, :])
                vF = gp.tile([128, CH2], dt, tag="v")
                nc.vector.tensor_mul(vF[H:128, :], rzF[H:128, :], uF[H:128, :])
                dst = st["hf"] if s == nsteps else st["nxt"]
                ew.tensor_add(dst[H:128, :], ntF[H:128, :], vF[H:128, :])
                st["cur"], st["nxt"] = dst, st["cur"]

            # --- GRU loops: PIPE pairs interleaved ---
            for base in range(0, NP, PIPE):
                grp = [start_pair(p) for p in range(base, min(base + PIPE, NP))]
                maxs = max(pair_steps[st["p"]] for st in grp)
                for s in range(1, maxs + 1):
                    for st in grp:
                        if s <= pair_steps[st["p"]]:
                            emit_step(st, s)

            # --- MLP heads, two barrier-separated phases so ACT runs all its
            # Exp calls together (one table swap) then all Sigmoids ---
            tc.no_sync_barrier()
            f2rhs = {}
            for p in range(NP):
                for g in (0, 1):
                    gs = slice(g * CH, (g + 1) * CH)
                    cur_g = hfin[p][H:128, gs]
                    for j in range(FC // 128):
                        pfF = psA.tile([128, CH2], _f32, tag="ps")
                        pf = pfF[:, 0:CH]
                        nc.tensor.matmul(
                            pf, w1F[H:128, j * 128 : (j + 1) * 128], cur_g,
                            start=True, stop=True,
                        )
                        # elu(x+b1) + 1 == (x+b1 - m) + exp(m),  m = min(x+b1, 0)
                        m = fp.tile([128, CH], dt, tag="m")
                        nc.vector.tensor_scalar(
                            m[:], pf, b1s[:, j : j + 1], 0.0, op0=OP.add, op1=OP.min
                        )
                        e = fp.tile([128, CH], dt, tag="e")
                        nc.scalar.activation(e[:], m[:], AF.Exp)
                        pp = fp.tile([128, CH], dt, tag="pp")
                        nc.vector.scalar_tensor_tensor(
                            pp[:], pf, b1s[:, j : j + 1], m[:],
                            op0=OP.add, op1=OP.subtract,
                        )
                        fr = frp.tile([128, CH], dt, tag=f"fr{p}{g}{j}")
                        nc.vector.tensor_add(fr[:], pp[:], e[:])
                        f2rhs[(p, g, j)] = fr

            tc.no_sync_barrier()
            for p in range(NP):
                for g in (0, 1):
                    for mt in range(OTILES):
                        mw = min(128, OUT - mt * 128)
                        poF = psB.tile([128, CH2], _f32, tag="pn")
                        po = poF[0:mw, 0:CH]
                        nc.tensor.matmul(
                            po, w2a[:, mt * 128 : mt * 128 + mw],
                            f2rhs[(p, g, 0)][:], start=True, stop=False,
                        )
                        nc.tensor.matmul(
                            po, w2b[:, mt * 128 : mt * 128 + mw],
                            f2rhs[(p, g, 1)][:], start=False, stop=True,
                        )
                        ot = op_.tile([mw, CH], _f32, tag="ot")
                        nc.scalar.activation(
                            ot[:], po, AF.Sigmoid, bias=b2s[0:mw, mt : mt + 1]
                        )
                        nc.sync.dma_start(
                            out=OT[mt * 128 : mt * 128 + mw, p * CH2 + g * CH :
                                   p * CH2 + (g + 1) * CH],
                            in_=ot[:],
                        )

    nc.compile()
    return nc


def kernel(message, lengths, init_emb, W_emb, b_emb, W_ih, W_hh, b_ih, b_hh,
           W1, b1, W2, b2):
    global LAST_RESULT
    message = np.asarray(message, dtype=np.float32)
    lengths = np.asarray(lengths).astype(np.int64)
    f8 = np.float64
    np_dt = ml_dtypes.bfloat16 if USE_BF16 else np.float32
    dt = _bf16 if USE_BF16 else _f32

    # --- fold embedding into input weights;  step 0 is a constant ---
    W_c = W_ih.astype(f8) @ W_emb.astype(f8)                # [3H, V]
    b_c = W_ih.astype(f8) @ b_emb.astype(f8) + b_ih         # [3H]
    gx0 = W_ih.astype(f8) @ init_emb.astype(f8) + b_ih
    gh0 = b_hh.astype(f8)
    r0 = _sigmoid(gx0[:H] + gh0[:H])
    z0 = _sigmoid(gx0[H : 2 * H] + gh0[H : 2 * H])
    n0 = np.tanh(gx0[2 * H :] + r0 * gh0[2 * H :])
    h1 = (1.0 - z0) * n0                                    # h after step 0

    # --- length-sort, stratify across cores ---
    perm = np.argsort(lengths, kind="stable")
    lsort = lengths[perm]
    # pair p (on every core) covers global sorted ranks [p*8192, (p+1)*8192)
    pair_steps = [int(lsort[min((p + 1) * CH2 * NCORES, B) - 1]) - 1
                  for p in range(NP)]

    # --- shared weight tensors ---
    WXh = np.zeros((KX, 3 * H), f8)
    WXh[:V] = W_c.T
    WXh[V, H : 2 * H] = FREEZE
    wxd = WXh.astype(np_dt)
    whd = np.ascontiguousarray(W_hh.T).astype(np_dt)
    w1d = np.ascontiguousarray(W1.T).astype(np_dt)
    w2d = np.ascontiguousarray(W2.T).astype(np_dt)
    brzd = np.ascontiguousarray((b_c[: 2 * H] + b_hh[: 2 * H]).astype(np.float32).reshape(2 * H, 1))
    bhnd = np.ascontiguousarray(b_hh[2 * H :].astype(np.float32).reshape(H, 1))
    bcnd = np.ascontiguousarray(b_c[2 * H :].astype(np.float32).reshape(H, 1))
    b1d = np.ascontiguousarray(np.asarray(b1, np.float32).reshape(FC // 128, 128).T)
    b2f = (np.asarray(b2, f8) - W2.astype(f8).sum(axis=1)).astype(np.float32)
    b2p = np.zeros(OTILES * 128, np.float32)
    b2p[:OUT] = b2f
    b2d = np.ascontiguousarray(b2p.reshape(OTILES, 128).T)
    h1d = np.ascontiguousarray(h1.astype(np.float32).reshape(H, 1))

    # --- per-core inputs ---
    trange = np.arange(T - 1)
    in_maps = []
    core_idx = []
    for c in range(NCORES):
        ic = perm[c::NCORES]
        core_idx.append(ic)
        mc = message[ic][:, : T - 1, :]                     # [BC, 29, 21]
        Xc = np.empty((T - 1, KX, BC), dtype=np_dt)
        Xc[:, :V, :] = mc.transpose(1, 2, 0).astype(np_dt)
        Xc[:, V, :] = (lengths[ic][None, :] <= trange[:, None] + 1).astype(np_dt)
        in_maps.append({
            "X": Xc, "WX": wxd, "WH": whd, "W1T": w1d, "W2T": w2d,
            "BRZ": brzd, "BHN": bhnd, "BCN": bcnd, "B1": b1d, "B2": b2d,
            "H1": h1d,
        })

    nc = _build_nc(pair_steps, dt)
    res = run_bass_kernel_spmd(nc, in_maps, core_ids=list(range(NCORES)), trace=TRACE)
    LAST_RESULT = res

    out = np.empty((B, OUT), np.float32)
    for c in range(NCORES):
        out[core_idx[c]] = res.results[c]["OT"].T
    return out


# revision 18
# speedup vs baseline: 1.3608x; 1.0413x over previous
"""Trainium2 Bass kernel for nn_Decoder (GRU decoder over padded sequences).

Computation (per sample):
  emb = message[:, :T-1] @ W_emb.T + b_emb            (folded into W_c on host)
  xs  = [init_emb, emb]                                (step 0 folded into h1 const)
  GRU over T steps, gather h at lengths-1              (freeze trick: z := 1 past len)
  out = sigmoid(elu(h @ W1.T + b1) @ W2.T + b2)

Sharding: batch data-parallel over 8 cores, host-side length-sort (stratified
round-robin across cores) so each chunk only runs to its max length.

Device layout: features on partitions, batch on the free dim. The GRU state h
(and the weights it multiplies) live at partition base 64 so that the z-gate
slice of the sigmoid output tile (partitions 64..127) is base-aligned with the
h-update operands — every elementwise op's SBUF operands share a start
partition, which the ISA requires. Chunks are processed in pairs (1024
samples) with the h-update ops fused at FD=1024, and pairs are interleaved
PIPE at a time so the serial per-step dependency chain overlaps.
"""

import sys

sys.path.insert(0, "/opt/trn_rl_repo")

import numpy as np
import ml_dtypes

import concourse.bacc as bacc
import concourse.mybir as mybir
import concourse.tile as tile
from concourse.bass_utils import run_bass_kernel_spmd

B, T, V, E, H, FC, OUT = 65536, 30, 21, 32, 64, 256, 784
NCORES = 8
BC = B // NCORES          # 8192 samples per core
CH = 512                  # matmul free dim (one PSUM bank)
CH2 = 2 * CH              # pair width
NP = BC // CH2            # 8 pairs per core
FREEZE = 40.0             # z-gate preactivation offset for finished samples
KX = V + 1                # 21 msg rows + 1 freeze-flag row
OTILES = (OUT + 127) // 128  # 7 output row tiles
PIPE = 2                  # pairs in flight
XB = 8                    # GRU steps per streamed X block

USE_BF16 = True
GP_HP = True              # final h-update add on GpSimd (else VectorE)
HN_COPY = True            # stage hn PSUM->SBUF via ScalarE so t1 runs at 2x
TRACE = False             # set by test harness for profiling
LAST_RESULT = None        # BassKernelResults stash for the harness

_f32 = mybir.dt.float32
_bf16 = mybir.dt.bfloat16


def _sigmoid(x):
    return 1.0 / (1.0 + np.exp(-x))


def _build_nc(pair_steps, dt):
    """Build the SPMD kernel. pair_steps[p] = GRU steps (beyond the constant
    step 0) for pair p — identical on every core."""
    AF = mybir.ActivationFunctionType
    OP = mybir.AluOpType
    nc = bacc.Bacc("TRN2", target_bir_lowering=False, debug=False)

    X = nc.dram_tensor("X", [T - 1, KX, BC], dt, kind="ExternalInput")
    WX = nc.dram_tensor("WX", [KX, 3 * H], dt, kind="ExternalInput")
    WH = nc.dram_tensor("WH", [H, 3 * H], dt, kind="ExternalInput")
    W1T = nc.dram_tensor("W1T", [H, FC], dt, kind="ExternalInput")
    W2T = nc.dram_tensor("W2T", [FC, OUT], dt, kind="ExternalInput")
    BRZ = nc.dram_tensor("BRZ", [2 * H, 1], _f32, kind="ExternalInput")
    BHN = nc.dram_tensor("BHN", [H, 1], _f32, kind="ExternalInput")
    BCN = nc.dram_tensor("BCN", [H, 1], _f32, kind="ExternalInput")
    B1 = nc.dram_tensor("B1", [128, FC // 128], _f32, kind="ExternalInput")
    B2 = nc.dram_tensor("B2", [128, OTILES], _f32, kind="ExternalInput")
    H1 = nc.dram_tensor("H1", [H, 1], _f32, kind="ExternalInput")
    OT = nc.dram_tensor("OT", [OUT, BC], _f32, kind="ExternalOutput")

    ew = nc.gpsimd if GP_HP else nc.vector

    with tile.TileContext(nc) as tc:
        with (
            tc.tile_pool(name="weights", bufs=1) as wp,
            tc.tile_pool(name="xin", bufs=2 * PIPE) as xp,
            tc.tile_pool(name="hstate", bufs=2 * PIPE) as hp,
            tc.tile_pool(name="hfinal", bufs=1) as hf,
            tc.tile_pool(name="gates", bufs=3) as gp,
            tc.tile_pool(name="head", bufs=3) as fp,
            tc.tile_pool(name="frhs", bufs=1) as frp,
            tc.tile_pool(name="outs", bufs=3) as op_,
            tc.tile_pool(name="psA", bufs=2, space="PSUM") as psA,
            tc.tile_pool(name="psB", bufs=2, space="PSUM") as psB,
        ):
            # --- load weights/biases once; h-side weights live at base 64 ---
            wx = wp.tile([KX, 3 * H], dt)
            nc.sync.dma_start(out=wx[:], in_=WX[:])
            whF = wp.tile([128, 3 * H], dt)
            nc.sync.dma_start(out=whF[H:128, :], in_=WH[:])
            w1F = wp.tile([128, FC], dt)
            nc.sync.dma_start(out=w1F[H:128, :], in_=W1T[:])
            w2a = wp.tile([128, OUT], dt)
            nc.sync.dma_start(out=w2a[:], in_=W2T[0:128, :])
            w2b = wp.tile([128, OUT], dt)
            nc.sync.dma_start(out=w2b[:], in_=W2T[128:256, :])
            brz = wp.tile([2 * H, 1], _f32)
            nc.sync.dma_start(out=brz[:], in_=BRZ[:])
            bhn = wp.tile([H, 1], _f32)
            nc.sync.dma_start(out=bhn[:], in_=BHN[:])
            bcn = wp.tile([H, 1], _f32)
            nc.sync.dma_start(out=bcn[:], in_=BCN[:])
            b1s = wp.tile([128, FC // 128], _f32)
            nc.sync.dma_start(out=b1s[:], in_=B1[:])
            b2s = wp.tile([128, OTILES], _f32)
            nc.sync.dma_start(out=b2s[:], in_=B2[:])
            h1F = wp.tile([128, 1], _f32)
            nc.sync.dma_start(out=h1F[H:128, :], in_=H1[:])

            wh = whF[H:128, :]
            hfin = []

            # PE warm-up: ~5us of dense junk matmuls so HAM un-throttles the
            # clock gate (4/8 -> 8/8) before the GRU loops start.
            warm = psA.tile([128, CH2], _f32, tag="ps")
            for _ in range(30):
                nc.tensor.matmul(
                    warm[:, 0 : 3 * H], wx[:, 0:128], wx[:], start=True, stop=True
                )

            def start_pair(p):
                """Allocate tiles + init h for pair p; returns state dict."""
                ha = hp.tile([128, CH2], dt, tag="hpi")
                hb = hp.tile([128, CH2], dt, tag="hpo")
                hfp = hf.tile([128, CH2], dt, tag=f"hf{p}")
                hfin.append(hfp)
                nc.vector.memset(ha[H:128, :], 0.0)
                nc.vector.tensor_scalar_add(ha[H:128, :], ha[H:128, :], h1F[H:128, :])
                if pair_steps[p] == 0:
                    nc.vector.memset(hfp[H:128, :], 0.0)
                    nc.vector.tensor_scalar_add(
                        hfp[H:128, :], hfp[H:128, :], h1F[H:128, :]
                    )
                return {"xt": None, "cur": ha, "nxt": hb, "hf": hfp, "p": p}

            def emit_step(st, s):
                """One GRU step for both chunks of a pair."""
                cur = st["cur"]
                nsteps = pair_steps[st["p"]]
                if (s - 1) % XB == 0:
                    nb = min(XB, nsteps - (s - 1))
                    xtn = xp.tile([KX, XB, CH2], dt, tag="xt")
                    st["xt"] = xtn
                    ps = slice(st["p"] * CH2, (st["p"] + 1) * CH2)
                    nc.sync.dma_start(
                        out=st["xt"][:, 0:nb, :],
                        in_=X[s - 1 : s - 1 + nb, :, ps].rearrange("t k b -> k t b"),
                    )
                xt = st["xt"]
                # 2-bank PSUM tiles: both chunks side by side -> FD=1024 ops.
                # pn layout: hn on partitions 0..63, xn on partitions 64..127.
                prz2 = psA.tile([128, CH2], _f32, tag="ps")
                pn2 = psB.tile([128, CH2], _f32, tag="pn")
                for g in (0, 1):
                    gs = slice(g * CH, (g + 1) * CH)
                    xs_ = xt[:, (s - 1) % XB, gs]
                    cur_g = cur[H:128, gs]
                    nc.tensor.matmul(prz2[:, gs], wx[:, 0:128], xs_, start=True, stop=False)
                    nc.tensor.matmul(prz2[:, gs], wh[:, 0:128], cur_g, start=False, stop=True)
                    nc.tensor.matmul(pn2[0:H, gs], wh[:, 128:192], cur_g, start=True, stop=True)
                    nc.tensor.matmul(pn2[H:128, gs], wx[:, 128:192], xs_, start=True, stop=True)
                rzF = gp.tile([128, CH2], dt, tag="rz")
                nc.scalar.activation(rzF[:], prz2[:], AF.Sigmoid, bias=brz[:])
                t1 = gp.tile([H, CH2], dt, tag="t1")
                if HN_COPY:
                    # hns = hn + b_hn via ACT (bias folded into the PSUM move),
                    # then t1 = hns * r as an all-SBUF bf16 TT (2x mode).
                    hns = gp.tile([H, CH2], dt, tag="hns")
                    nc.scalar.activation(
                        hns[:], pn2[0:H, :], AF.Identity, bias=bhn[:]
                    )
                    nc.vector.tensor_mul(t1[:], hns[:], rzF[0:H, :])
                else:
                    nc.vector.scalar_tensor_tensor(
                        t1[:], pn2[0:H, :], bhn[:], rzF[0:H, :], op0=OP.add, op1=OP.mult
                    )
                # (xn + b_cn) + t1   (xn read from PSUM at base 64: exempt)
                t2F = gp.tile([H, CH2], dt, tag="t2")
                nc.vector.scalar_tensor_tensor(
                    t2F[:], pn2[H:128, :], bcn[:], t1[:], op0=OP.add, op1=OP.add
                )
                # n = tanh(t2);  h' = n + z*(h - n)
                ntF = gp.tile([128, CH2], dt, tag="nt")
                nc.scalar.activation(ntF[H:128, :], t2F[:], AF.Tanh)
                uF = gp.tile([128, CH2], dt, tag="u")
                nc.vector.tensor_sub(uF[H:128, :], cur[H:128, :], ntF[H:128, :])
                vF = gp.tile([128, CH2], dt, tag="v")
                nc.vector.tensor_mul(vF[H:128, :], rzF[H:128, :], uF[H:128, :])
                dst = st["hf"] if s == nsteps else st["nxt"]
                ew.tensor_add(dst[H:128, :], ntF[H:128, :], vF[H:128, :])
                st["cur"], st["nxt"] = dst, st["cur"]

            # --- GRU loops: PIPE pairs interleaved ---
            for base in range(0, NP, PIPE):
                grp = [start_pair(p) for p in range(base, min(base + PIPE, NP))]
                maxs = max(pair_steps[st["p"]] for st in grp)
                for s in range(1, maxs + 1):
                    for st in grp:
                        if s <= pair_steps[st["p"]]:
                            emit_step(st, s)

            # --- MLP heads, two barrier-separated phases so ACT runs all its
            # Exp calls together (one table swap) then all Sigmoids ---
            tc.no_sync_barrier()
            f2rhs = {}
            for p in range(NP):
                for g in (0, 1):
                    gs = slice(g * CH, (g + 1) * CH)
                    cur_g = hfin[p][H:128, gs]
                    for j in range(FC // 128):
                        pfF = psA.tile([128, CH2], _f32, tag="ps")
                        pf = pfF[:, 0:CH]
                        nc.tensor.matmul(
                            pf, w1F[H:128, j * 128 : (j + 1) * 128], cur_g,
                            start=True, stop=True,
                        )
                        # elu(x+b1) + 1 == (x+b1 - m) + exp(m),  m = min(x+b1, 0)
                        m = fp.tile([128, CH], dt, tag="m")
                        nc.vector.tensor_scalar(
                            m[:], pf, b1s[:, j : j + 1], 0.0, op0=OP.add, op1=OP.min
                        )
                        e = fp.tile([128, CH], dt, tag="e")
                        nc.scalar.activation(e[:], m[:], AF.Exp)
                        pp = fp.tile([128, CH], dt, tag="pp")
                        nc.vector.scalar_tensor_tensor(
                            pp[:], pf, b1s[:, j : j + 1], m[:],
                            op0=OP.add, op1=OP.subtract,
                        )
                        fr = frp.tile([128, CH], dt, tag=f"fr{p}{g}{j}")
                        nc.vector.tensor_add(fr[:], pp[:], e[:])
                        f2rhs[(p, g, j)] = fr

            tc.no_sync_barrier()
            for p in range(NP):
                for g in (0, 1):
                    for mt in range(OTILES):
                        mw = min(128, OUT - mt * 128)
                        poF = psB.tile([128, CH2], _f32, tag="pn")
                        po = poF[0:mw, 0:CH]
                        nc.tensor.matmul(
                            po, w2a[:, mt * 128 : mt * 128 + mw],
                            f2rhs[(p, g, 0)][:], start=True, stop=False,
                        )
                        nc.tensor.matmul(
                            po, w2b[:, mt * 128 : mt * 128 + mw],
                            f2rhs[(p, g, 1)][:], start=False, stop=True,
                        )
                        ot = op_.tile([mw, CH], _f32, tag="ot")
                        nc.scalar.activation(
                            ot[:], po, AF.Sigmoid, bias=b2s[0:mw, mt : mt + 1]
                        )
                        nc.sync.dma_start(
                            out=OT[mt * 128 : mt * 128 + mw, p * CH2 + g * CH :
                                   p * CH2 + (g + 1) * CH],
                            in_=ot[:],
                        )

    nc.compile()
    return nc


def kernel(message, lengths, init_emb, W_emb, b_emb, W_ih, W_hh, b_ih, b_hh,
           W1, b1, W2, b2):
    global LAST_RESULT
    message = np.asarray(message, dtype=np.float32)
    lengths = np.asarray(lengths).astype(np.int64)
    f8 = np.float64
    np_dt = ml_dtypes.bfloat16 if USE_BF16 else np.float32
    dt = _bf16 if USE_BF16 else _f32

    # --- fold embedding into input weights;  step 0 is a constant ---
    W_c = W_ih.astype(f8) @ W_emb.astype(f8)                # [3H, V]
    b_c = W_ih.astype(f8) @ b_emb.astype(f8) + b_ih         # [3H]
    gx0 = W_ih.astype(f8) @ init_emb.astype(f8) + b_ih
    gh0 = b_hh.astype(f8)
    r0 = _sigmoid(gx0[:H] + gh0[:H])
    z0 = _sigmoid(gx0[H : 2 * H] + gh0[H : 2 * H])
    n0 = np.tanh(gx0[2 * H :] + r0 * gh0[2 * H :])
    h1 = (1.0 - z0) * n0                                    # h after step 0

    # --- length-sort, stratify across cores ---
    perm = np.argsort(lengths, kind="stable")
    lsort = lengths[perm]
    # pair p (on every core) covers global sorted ranks [p*8192, (p+1)*8192)
    pair_steps = [int(lsort[min((p + 1) * CH2 * NCORES, B) - 1]) - 1
                  for p in range(NP)]

    # --- shared weight tensors ---
    WXh = np.zeros((KX, 3 * H), f8)
    WXh[:V] = W_c.T
    WXh[V, H : 2 * H] = FREEZE
    wxd = WXh.astype(np_dt)
    whd = np.ascontiguousarray(W_hh.T).astype(np_dt)
    w1d = np.ascontiguousarray(W1.T).astype(np_dt)
    w2d = np.ascontiguousarray(W2.T).astype(np_dt)
    brzd = np.ascontiguousarray((b_c[: 2 * H] + b_hh[: 2 * H]).astype(np.float32).reshape(2 * H, 1))
    bhnd = np.ascontiguousarray(b_hh[2 * H :].astype(np.float32).reshape(H, 1))
    bcnd = np.ascontiguousarray(b_c[2 * H :].astype(np.float32).reshape(H, 1))
    b1d = np.ascontiguousarray(np.asarray(b1, np.float32).reshape(FC // 128, 128).T)
    b2f = (np.asarray(b2, f8) - W2.astype(f8).sum(axis=1)).astype(np.float32)
    b2p = np.zeros(OTILES * 128, np.float32)
    b2p[:OUT] = b2f
    b2d = np.ascontiguousarray(b2p.reshape(OTILES, 128).T)
    h1d = np.ascontiguousarray(h1.astype(np.float32).reshape(H, 1))

    # --- per-core inputs ---
    trange = np.arange(T - 1)
    in_maps = []
    core_idx = []
    for c in range(NCORES):
        ic = perm[c::NCORES]
        core_idx.append(ic)
        mc = message[ic][:, : T - 1, :]                     # [BC, 29, 21]
        Xc = np.empty((T - 1, KX, BC), dtype=np_dt)
        Xc[:, :V, :] = mc.transpose(1, 2, 0).astype(np_dt)
        Xc[:, V, :] = (lengths[ic][None, :] <= trange[:, None] + 1).astype(np_dt)
        in_maps.append({
            "X": Xc, "WX": wxd, "WH": whd, "W1T": w1d, "W2T": w2d,
            "BRZ": brzd, "BHN": bhnd, "BCN": bcnd, "B1": b1d, "B2": b2d,
            "H1": h1d,
        })

    nc = _build_nc(pair_steps, dt)
    res = run_bass_kernel_spmd(nc, in_maps, core_ids=list(range(NCORES)), trace=TRACE)
    LAST_RESULT = res

    out = np.empty((B, OUT), np.float32)
    for c in range(NCORES):
        out[core_idx[c]] = res.results[c]["OT"].T
    return out


# revision 19
# speedup vs baseline: 2.0356x; 1.4960x over previous
"""Trainium2 Bass kernel for nn_Decoder (GRU decoder over padded sequences).

Computation (per sample):
  emb = message[:, :T-1] @ W_emb.T + b_emb            (folded into W_c on host)
  xs  = [init_emb, emb]                                (step 0 folded into h1 const)
  GRU over T steps, gather h at lengths-1              (freeze trick: z := 1 past len)
  out = sigmoid(elu(h @ W1.T + b1) @ W2.T + b2)

Sharding: batch data-parallel over 8 cores, host-side length-sort (stratified
round-robin across cores) so each chunk only runs to its max length.

Device layout: features on partitions, batch on the free dim. The GRU state h
(and the weights it multiplies) live at partition base 64 so that the z-gate
slice of the sigmoid output tile (partitions 64..127) is base-aligned with the
h-update operands — every elementwise op's SBUF operands share a start
partition, which the ISA requires. Chunks are processed in pairs (1024
samples) with the h-update ops fused at FD=1024, and pairs are interleaved
PIPE at a time so the serial per-step dependency chain overlaps.
"""

import sys

sys.path.insert(0, "/opt/trn_rl_repo")

import numpy as np
import ml_dtypes

import concourse.bacc as bacc
import concourse.mybir as mybir
import concourse.tile as tile
from concourse.bass_utils import run_bass_kernel_spmd

B, T, V, E, H, FC, OUT = 65536, 30, 21, 32, 64, 256, 784
NCORES = 8
BC = B // NCORES          # 8192 samples per core
CH = 512                  # matmul free dim (one PSUM bank)
CH2 = 2 * CH              # pair width
NP = BC // CH2            # 8 pairs per core
FREEZE = 40.0             # z-gate preactivation offset for finished samples
KX = V + 1                # 21 msg rows + 1 freeze-flag row
OTILES = (OUT + 127) // 128  # 7 output row tiles
PIPE = 8                  # pairs in flight (all interleaved)
XB = 2                    # GRU steps per streamed X block

USE_BF16 = True
GP_HP = False             # final h-update add on GpSimd (else VectorE)
HN_COPY = True            # stage hn PSUM->SBUF via ScalarE so t1 runs at 2x
TRACE = False             # set by test harness for profiling
LAST_RESULT = None        # BassKernelResults stash for the harness

_f32 = mybir.dt.float32
_bf16 = mybir.dt.bfloat16


def _sigmoid(x):
    return 1.0 / (1.0 + np.exp(-x))


def _build_nc(pair_steps, dt):
    """Build the SPMD kernel. pair_steps[p] = GRU steps (beyond the constant
    step 0) for pair p — identical on every core."""
    AF = mybir.ActivationFunctionType
    OP = mybir.AluOpType
    nc = bacc.Bacc("TRN2", target_bir_lowering=False, debug=False)

    X = nc.dram_tensor("X", [T - 1, KX, BC], dt, kind="ExternalInput")
    WX = nc.dram_tensor("WX", [KX, 3 * H], dt, kind="ExternalInput")
    WH = nc.dram_tensor("WH", [H, 3 * H], dt, kind="ExternalInput")
    W1T = nc.dram_tensor("W1T", [H, FC], dt, kind="ExternalInput")
    W2T = nc.dram_tensor("W2T", [FC, OUT], dt, kind="ExternalInput")
    BRZ = nc.dram_tensor("BRZ", [2 * H, 1], _f32, kind="ExternalInput")
    BHN = nc.dram_tensor("BHN", [H, 1], _f32, kind="ExternalInput")
    BCN = nc.dram_tensor("BCN", [H, 1], _f32, kind="ExternalInput")
    B1 = nc.dram_tensor("B1", [128, FC // 128], _f32, kind="ExternalInput")
    B2 = nc.dram_tensor("B2", [128, OTILES], _f32, kind="ExternalInput")
    H1 = nc.dram_tensor("H1", [H, 1], _f32, kind="ExternalInput")
    OT = nc.dram_tensor("OT", [OUT, BC], _f32, kind="ExternalOutput")

    ew = nc.gpsimd if GP_HP else nc.vector

    with tile.TileContext(nc) as tc:
        with (
            tc.tile_pool(name="weights", bufs=1) as wp,
            tc.tile_pool(name="xin", bufs=9) as xp,
            tc.tile_pool(name="hstate", bufs=NP) as hp,
            tc.tile_pool(name="hfinal", bufs=1) as hf,
            tc.tile_pool(name="gates", bufs=3) as gp,
            tc.tile_pool(name="head", bufs=3) as fp,
            tc.tile_pool(name="frhs", bufs=1) as frp,
            tc.tile_pool(name="outs", bufs=3) as op_,
            tc.tile_pool(name="psA", bufs=2, space="PSUM") as psA,
            tc.tile_pool(name="psB", bufs=2, space="PSUM") as psB,
        ):
            # --- load weights/biases once; h-side weights live at base 64 ---
            wx = wp.tile([KX, 3 * H], dt)
            nc.sync.dma_start(out=wx[:], in_=WX[:])
            whF = wp.tile([128, 3 * H], dt)
            nc.sync.dma_start(out=whF[H:128, :], in_=WH[:])
            w1F = wp.tile([128, FC], dt)
            nc.sync.dma_start(out=w1F[H:128, :], in_=W1T[:])
            w2a = wp.tile([128, OUT], dt)
            nc.sync.dma_start(out=w2a[:], in_=W2T[0:128, :])
            w2b = wp.tile([128, OUT], dt)
            nc.sync.dma_start(out=w2b[:], in_=W2T[128:256, :])
            brz = wp.tile([2 * H, 1], _f32)
            nc.sync.dma_start(out=brz[:], in_=BRZ[:])
            bhn = wp.tile([H, 1], _f32)
            nc.sync.dma_start(out=bhn[:], in_=BHN[:])
            bcn = wp.tile([H, 1], _f32)
            nc.sync.dma_start(out=bcn[:], in_=BCN[:])
            b1s = wp.tile([128, FC // 128], _f32)
            nc.sync.dma_start(out=b1s[:], in_=B1[:])
            b2s = wp.tile([128, OTILES], _f32)
            nc.sync.dma_start(out=b2s[:], in_=B2[:])
            h1F = wp.tile([128, 1], _f32)
            nc.sync.dma_start(out=h1F[H:128, :], in_=H1[:])

            wh = whF[H:128, :]
            hfin = []

            # PE warm-up: ~5us of dense junk matmuls so HAM un-throttles the
            # clock gate (4/8 -> 8/8) before the GRU loops start.
            warm = psA.tile([128, CH2], _f32, tag="ps")
            for _ in range(30):
                nc.tensor.matmul(
                    warm[:, 0 : 3 * H], wx[:, 0:128], wx[:], start=True, stop=True
                )

            def start_pair(p):
                """Allocate tiles + init h for pair p; returns state dict."""
                ha = hp.tile([128, CH2], dt, tag="hpi")
                hb = hp.tile([128, CH2], dt, tag="hpo")
                hfp = hf.tile([128, CH2], dt, tag=f"hf{p}")
                hfin.append(hfp)
                nc.vector.memset(ha[H:128, :], 0.0)
                nc.vector.tensor_scalar_add(ha[H:128, :], ha[H:128, :], h1F[H:128, :])
                if pair_steps[p] == 0:
                    nc.vector.memset(hfp[H:128, :], 0.0)
                    nc.vector.tensor_scalar_add(
                        hfp[H:128, :], hfp[H:128, :], h1F[H:128, :]
                    )
                return {"xt": None, "cur": ha, "nxt": hb, "hf": hfp, "p": p}

            def emit_step(st, s):
                """One GRU step for both chunks of a pair."""
                cur = st["cur"]
                nsteps = pair_steps[st["p"]]
                if (s - 1) % XB == 0:
                    nb = min(XB, nsteps - (s - 1))
                    xtn = xp.tile([KX, XB, CH2], dt, tag="xt")
                    st["xt"] = xtn
                    ps = slice(st["p"] * CH2, (st["p"] + 1) * CH2)
                    nc.sync.dma_start(
                        out=st["xt"][:, 0:nb, :],
                        in_=X[s - 1 : s - 1 + nb, :, ps].rearrange("t k b -> k t b"),
                    )
                xt = st["xt"]
                # 2-bank PSUM tiles: both chunks side by side -> FD=1024 ops.
                # pn layout: hn on partitions 0..63, xn on partitions 64..127.
                prz2 = psA.tile([128, CH2], _f32, tag="ps")
                pn2 = psB.tile([128, CH2], _f32, tag="pn")
                for g in (0, 1):
                    gs = slice(g * CH, (g + 1) * CH)
                    xs_ = xt[:, (s - 1) % XB, gs]
                    cur_g = cur[H:128, gs]
                    nc.tensor.matmul(prz2[:, gs], wx[:, 0:128], xs_, start=True, stop=False)
                    nc.tensor.matmul(prz2[:, gs], wh[:, 0:128], cur_g, start=False, stop=True)
                    nc.tensor.matmul(pn2[0:H, gs], wh[:, 128:192], cur_g, start=True, stop=True)
                    nc.tensor.matmul(pn2[H:128, gs], wx[:, 128:192], xs_, start=True, stop=True)
                rzF = gp.tile([128, CH2], dt, tag="rz")
                nc.scalar.activation(rzF[:], prz2[:], AF.Sigmoid, bias=brz[:])
                t1 = gp.tile([H, CH2], dt, tag="t1")
                if HN_COPY:
                    # hns = hn + b_hn via ACT (bias folded into the PSUM move),
                    # then t1 = hns * r as an all-SBUF bf16 TT (2x mode).
                    hns = gp.tile([H, CH2], dt, tag="hns")
                    nc.scalar.activation(
                        hns[:], pn2[0:H, :], AF.Identity, bias=bhn[:]
                    )
                    nc.vector.tensor_mul(t1[:], hns[:], rzF[0:H, :])
                else:
                    nc.vector.scalar_tensor_tensor(
                        t1[:], pn2[0:H, :], bhn[:], rzF[0:H, :], op0=OP.add, op1=OP.mult
                    )
                # (xn + b_cn) + t1   (xn read from PSUM at base 64: exempt)
                t2F = gp.tile([H, CH2], dt, tag="t2")
                nc.vector.scalar_tensor_tensor(
                    t2F[:], pn2[H:128, :], bcn[:], t1[:], op0=OP.add, op1=OP.add
                )
                # n = tanh(t2);  h' = n + z*(h - n)
                ntF = gp.tile([128, CH2], dt, tag="nt")
                nc.scalar.activation(ntF[H:128, :], t2F[:], AF.Tanh)
                uF = gp.tile([128, CH2], dt, tag="u")
                nc.vector.tensor_sub(uF[H:128, :], cur[H:128, :], ntF[H:128, :])
                vF = gp.tile([128, CH2], dt, tag="v")
                nc.vector.tensor_mul(vF[H:128, :], rzF[H:128, :], uF[H:128, :])
                dst = st["hf"] if s == nsteps else st["nxt"]
                ew.tensor_add(dst[H:128, :], ntF[H:128, :], vF[H:128, :])
                st["cur"], st["nxt"] = dst, st["cur"]

            # --- GRU loops: all pairs interleaved so short pairs' tails
            # overlap long pairs' matmul phases ---
            grp = [start_pair(p) for p in range(NP)]
            maxs = max(pair_steps[st["p"]] for st in grp)
            for s in range(1, maxs + 1):
                for st in grp:
                    if s <= pair_steps[st["p"]]:
                        emit_step(st, s)

            # --- MLP heads, two barrier-separated phases so ACT runs all its
            # Exp calls together (one table swap) then all Sigmoids ---
            tc.no_sync_barrier()
            f2rhs = {}
            for p in range(NP):
                for g in (0, 1):
                    gs = slice(g * CH, (g + 1) * CH)
                    cur_g = hfin[p][H:128, gs]
                    for j in range(FC // 128):
                        pfF = psA.tile([128, CH2], _f32, tag="ps")
                        pf = pfF[:, 0:CH]
                        nc.tensor.matmul(
                            pf, w1F[H:128, j * 128 : (j + 1) * 128], cur_g,
                            start=True, stop=True,
                        )
                        # elu(x+b1) + 1 == (x+b1 - m) + exp(m),  m = min(x+b1, 0)
                        m = fp.tile([128, CH], dt, tag="m")
                        nc.vector.tensor_scalar(
                            m[:], pf, b1s[:, j : j + 1], 0.0, op0=OP.add, op1=OP.min
                        )
                        e = fp.tile([128, CH], dt, tag="e")
                        nc.scalar.activation(e[:], m[:], AF.Exp)
                        pp = fp.tile([128, CH], dt, tag="pp")
                        nc.vector.scalar_tensor_tensor(
                            pp[:], pf, b1s[:, j : j + 1], m[:],
                            op0=OP.add, op1=OP.subtract,
                        )
                        fr = frp.tile([128, CH], dt, tag=f"fr{p}{g}{j}")
                        nc.vector.tensor_add(fr[:], pp[:], e[:])
                        f2rhs[(p, g, j)] = fr

            tc.no_sync_barrier()
            for p in range(NP):
                for g in (0, 1):
                    for mt in range(OTILES):
                        mw = min(128, OUT - mt * 128)
                        poF = psB.tile([128, CH2], _f32, tag="pn")
                        po = poF[0:mw, 0:CH]
                        nc.tensor.matmul(
                            po, w2a[:, mt * 128 : mt * 128 + mw],
                            f2rhs[(p, g, 0)][:], start=True, stop=False,
                        )
                        nc.tensor.matmul(
                            po, w2b[:, mt * 128 : mt * 128 + mw],
                            f2rhs[(p, g, 1)][:], start=False, stop=True,
                        )
                        ot = op_.tile([mw, CH], _f32, tag="ot")
                        nc.scalar.activation(
                            ot[:], po, AF.Sigmoid, bias=b2s[0:mw, mt : mt + 1]
                        )
                        nc.sync.dma_start(
                            out=OT[mt * 128 : mt * 128 + mw, p * CH2 + g * CH :
                                   p * CH2 + (g + 1) * CH],
                            in_=ot[:],
                        )

    nc.compile()
    return nc


def kernel(message, lengths, init_emb, W_emb, b_emb, W_ih, W_hh, b_ih, b_hh,
           W1, b1, W2, b2):
    global LAST_RESULT
    message = np.asarray(message, dtype=np.float32)
    lengths = np.asarray(lengths).astype(np.int64)
    f8 = np.float64
    np_dt = ml_dtypes.bfloat16 if USE_BF16 else np.float32
    dt = _bf16 if USE_BF16 else _f32

    # --- fold embedding into input weights;  step 0 is a constant ---
    W_c = W_ih.astype(f8) @ W_emb.astype(f8)                # [3H, V]
    b_c = W_ih.astype(f8) @ b_emb.astype(f8) + b_ih         # [3H]
    gx0 = W_ih.astype(f8) @ init_emb.astype(f8) + b_ih
    gh0 = b_hh.astype(f8)
    r0 = _sigmoid(gx0[:H] + gh0[:H])
    z0 = _sigmoid(gx0[H : 2 * H] + gh0[H : 2 * H])
    n0 = np.tanh(gx0[2 * H :] + r0 * gh0[2 * H :])
    h1 = (1.0 - z0) * n0                                    # h after step 0

    # --- length-sort, stratify across cores ---
    perm = np.argsort(lengths, kind="stable")
    lsort = lengths[perm]
    # pair p (on every core) covers global sorted ranks [p*8192, (p+1)*8192)
    pair_steps = [int(lsort[min((p + 1) * CH2 * NCORES, B) - 1]) - 1
                  for p in range(NP)]

    # --- shared weight tensors ---
    WXh = np.zeros((KX, 3 * H), f8)
    WXh[:V] = W_c.T
    WXh[V, H : 2 * H] = FREEZE
    wxd = WXh.astype(np_dt)
    whd = np.ascontiguousarray(W_hh.T).astype(np_dt)
    w1d = np.ascontiguousarray(W1.T).astype(np_dt)
    w2d = np.ascontiguousarray(W2.T).astype(np_dt)
    brzd = np.ascontiguousarray((b_c[: 2 * H] + b_hh[: 2 * H]).astype(np.float32).reshape(2 * H, 1))
    bhnd = np.ascontiguousarray(b_hh[2 * H :].astype(np.float32).reshape(H, 1))
    bcnd = np.ascontiguousarray(b_c[2 * H :].astype(np.float32).reshape(H, 1))
    b1d = np.ascontiguousarray(np.asarray(b1, np.float32).reshape(FC // 128, 128).T)
    b2f = (np.asarray(b2, f8) - W2.astype(f8).sum(axis=1)).astype(np.float32)
    b2p = np.zeros(OTILES * 128, np.float32)
    b2p[:OUT] = b2f
    b2d = np.ascontiguousarray(b2p.reshape(OTILES, 128).T)
    h1d = np.ascontiguousarray(h1.astype(np.float32).reshape(H, 1))

    # --- per-core inputs ---
    trange = np.arange(T - 1)
    in_maps = []
    core_idx = []
    for c in range(NCORES):
        ic = perm[c::NCORES]
        core_idx.append(ic)
        mc = message[ic][:, : T - 1, :]                     # [BC, 29, 21]
        Xc = np.empty((T - 1, KX, BC), dtype=np_dt)
        Xc[:, :V, :] = mc.transpose(1, 2, 0).astype(np_dt)
        Xc[:, V, :] = (lengths[ic][None, :] <= trange[:, None] + 1).astype(np_dt)
        in_maps.append({
            "X": Xc, "WX": wxd, "WH": whd, "W1T": w1d, "W2T": w2d,
            "BRZ": brzd, "BHN": bhnd, "BCN": bcnd, "B1": b1d, "B2": b2d,
            "H1": h1d,
        })

    nc = _build_nc(pair_steps, dt)
    res = run_bass_kernel_spmd(nc, in_maps, core_ids=list(range(NCORES)), trace=TRACE)
    LAST_RESULT = res

    out = np.empty((B, OUT), np.float32)
    for c in range(NCORES):
        out[core_idx[c]] = res.results[c]["OT"].T
    return out
